# revision 27
# baseline (speedup 1.0000x reference)
"""CLAHE (kornia-style) Trainium2 kernel, 8 NeuronCores, H-sharded.

Phase 1 (device): per-tile cnt_le histograms via fused compare+accumulate
  (DVE tensor_scalar is_le + ACT Sign, split across bins).
Phase 2 (device): bilinear LUT apply via GPSIMD ap_gather with a 4-variant
  table trick (idx = XQ*256 + bin), exact-order bilinear blend (DVE mults,
  PE 4-partition sum), exact floor pipeline.
Host: tiny (192,256) LUT math replicating the reference bit-exactly.
"""
import sys

sys.path.insert(0, "/opt/trn_rl_repo")
from contextlib import ExitStack

import numpy as np

import concourse.bass as bass
import concourse.tile as tile
from concourse import bacc, mybir
from concourse.bass_utils import run_bass_kernel_spmd

F32 = mybir.dt.float32
BF16 = mybir.dt.bfloat16
I16 = mybir.dt.int16
ALU = mybir.AluOpType
ACTF = mybir.ActivationFunctionType

N_CORES = 8
C, H, W = 3, 2048, 2048
GH = GW = 8
TH = TW = 256
NB = 256
RH = H // N_CORES          # rows per core = 256
FP = RH * 16               # free elems per partition per channel = 4096
MAGIC = float(2 ** 23)     # RNE-to-integer trick constant

# bins 0..254 computed on device; split between DVE and ACT
ACT_BIN_START = 194        # bins [194, 255) on ACT, [0, 194) on DVE
NUM_ELEMS = 9 * NB         # gather table entries (XQ in 0..8)


def _img_slice_ap(t, c):
    # dram img (C, RH, W) viewed as (128 part[colgroup], RH rows, 16 colin)
    return bass.AP(t, c * RH * W, [[16, 128], [W, RH], [1, 16]])


def build_phase1():
    nc = bacc.Bacc("TRN2", target_bir_lowering=False, debug=False,
                   num_devices=N_CORES)
    img = nc.dram_tensor("img", [C, RH, W], F32, kind="ExternalInput")
    abias = nc.dram_tensor("abias", [128, 64], F32, kind="ExternalInput")
    cntd = nc.dram_tensor("cntd", [C, 128, NB], F32, kind="ExternalOutput").ap()
    cnta = nc.dram_tensor("cnta", [C, 128, NB], F32, kind="ExternalOutput").ap()

    with tile.TileContext(nc) as tc, ExitStack() as ctx:
        vpool = ctx.enter_context(tc.tile_pool(name="v", bufs=2))
        tpool = ctx.enter_context(tc.tile_pool(name="t", bufs=7))
        bpool = ctx.enter_context(tc.tile_pool(name="b", bufs=2))
        jpool = ctx.enter_context(tc.tile_pool(name="j", bufs=2))
        cpool = ctx.enter_context(tc.tile_pool(name="c", bufs=2))
        kpool = ctx.enter_context(tc.tile_pool(name="k", bufs=1))
        ab = kpool.tile([128, 64], F32)
        nc.sync.dma_start(ab[:], abias.ap())
        for c in range(C):
            v = vpool.tile([128, FP], F32)
            nc.sync.dma_start(v[:], _img_slice_ap(img, c))
            # exact bins without divide: bins = #{b>=1: 256*v >= 255*b}
            # (min{v: fl(v/255) >= b/256} == 255b/256 exactly, host-verified)
            z = tpool.tile([128, FP], F32, tag="tmp")
            nc.vector.tensor_scalar(z[:], v[:], 256.0, None, ALU.mult)
            w = tpool.tile([128, FP], F32, tag="tmp")
            nc.vector.tensor_scalar(w[:], z[:], float(np.float32(1.0) / np.float32(255.0)), None, ALU.mult)
            y = tpool.tile([128, FP], F32, tag="tmp")
            nc.vector.tensor_scalar(y[:], w[:], MAGIC, MAGIC, ALU.add, ALU.subtract)
            g = tpool.tile([128, FP], F32, tag="tmp")
            nc.vector.tensor_tensor(g[:], y[:], w[:], ALU.is_gt)
            c0 = tpool.tile([128, FP], F32, tag="tmp")
            nc.vector.tensor_tensor(c0[:], y[:], g[:], ALU.subtract)
            m0 = tpool.tile([128, FP], F32, tag="tmp")
            nc.vector.tensor_scalar(m0[:], c0[:], 255.0, None, ALU.mult)
            s1 = tpool.tile([128, FP], F32, tag="tmp")
            nc.vector.tensor_scalar(s1[:], m0[:], 255.0, None, ALU.add)
            aa = tpool.tile([128, FP], F32, tag="tmp")
            nc.vector.tensor_tensor(aa[:], z[:], s1[:], ALU.is_ge)
            b2 = tpool.tile([128, FP], F32, tag="tmp")
            nc.vector.tensor_tensor(b2[:], z[:], m0[:], ALU.is_lt)
            tq = tpool.tile([128, FP], F32, tag="tmp")
            nc.vector.tensor_tensor(tq[:], aa[:], b2[:], ALU.subtract)
            f = tpool.tile([128, FP], F32, tag="tmp")
            nc.vector.tensor_tensor(f[:], c0[:], tq[:], ALU.add)
            bins = bpool.tile([128, FP], BF16)
            nc.vector.tensor_scalar(bins[:], f[:], 255.0, None, ALU.min)

            cd = cpool.tile([128, NB], F32, tag="cd")
            ca = cpool.tile([128, NB], F32, tag="ca")
            junkd = jpool.tile([128, FP], BF16, tag="jd")
            junka = jpool.tile([128, FP], BF16, tag="ja")
            for b in range(ACT_BIN_START):
                nc.vector.tensor_scalar(junkd[:], bins[:], b + 0.5, None,
                                        ALU.is_le, ALU.add,
                                        accum_out=cd[:, b:b + 1])
            for b in range(ACT_BIN_START, NB - 1):
                i = b - ACT_BIN_START
                nc.scalar.activation(junka[:], bins[:], ACTF.Sign,
                                     bias=ab[:, i:i + 1], scale=1.0,
                                     accum_out=ca[:, b:b + 1])
            nc.sync.dma_start(cntd[c][:, :ACT_BIN_START],
                              cd[:, :ACT_BIN_START])
            nc.sync.dma_start(cnta[c][:, ACT_BIN_START:NB - 1],
                              ca[:, ACT_BIN_START:NB - 1])
    nc.compile()
    return nc


def build_phase2():
    nc = bacc.Bacc("TRN2", target_bir_lowering=False, debug=False,
                   num_devices=N_CORES)
    img = nc.dram_tensor("img", [C, RH, W], F32, kind="ExternalInput")
    tab = nc.dram_tensor("tab", [2, C, 128, NUM_ELEMS], F32, kind="ExternalInput")
    wyt = nc.dram_tensor("wy", [2, 128, 128], F32, kind="ExternalInput")
    wxt = nc.dram_tensor("wx", [128, 256], F32, kind="ExternalInput")
    xqt = nc.dram_tensor("xq", [128, 1], F32, kind="ExternalInput")
    smt = nc.dram_tensor("sm", [128, 8], F32, kind="ExternalInput")
    out = nc.dram_tensor("out", [C, RH, W], F32, kind="ExternalOutput")
    scr = nc.dram_tensor("scr", [C, 2, 128, 2048], F32)  # output layout

    with tile.TileContext(nc) as tc, ExitStack() as ctx:
        ipool = ctx.enter_context(tc.tile_pool(name="i", bufs=1))
        kpool = ctx.enter_context(tc.tile_pool(name="k", bufs=1))

        # resident constants
        wy = kpool.tile([128, 2 * 128], F32)
        nc.sync.dma_start(wy[:], bass.AP(wyt, 0, [[128, 128], [16384, 2], [1, 128]]))
        wx = kpool.tile([128, 256], F32)
        nc.sync.dma_start(wx[:], wxt.ap())
        xq = kpool.tile([128, 1], F32)
        nc.sync.dma_start(xq[:], xqt.ap())
        # per-channel bins -> idx (int16), resident
        idxt = []
        with ExitStack() as prep:
            vpool = prep.enter_context(tc.tile_pool(name="v", bufs=2))
            tpool = prep.enter_context(tc.tile_pool(name="t", bufs=7))
            for c in range(C):
                v = vpool.tile([128, FP], F32)
                nc.sync.dma_start(v[:], _img_slice_ap(img, c))
                z = tpool.tile([128, FP], F32, tag="tmp")
                nc.vector.tensor_scalar(z[:], v[:], 256.0, None, ALU.mult)
                w = tpool.tile([128, FP], F32, tag="tmp")
                nc.vector.tensor_scalar(w[:], z[:], float(np.float32(1.0) / np.float32(255.0)), None, ALU.mult)
                y = tpool.tile([128, FP], F32, tag="tmp")
                nc.vector.tensor_scalar(y[:], w[:], MAGIC, MAGIC, ALU.add, ALU.subtract)
                g = tpool.tile([128, FP], F32, tag="tmp")
                nc.vector.tensor_tensor(g[:], y[:], w[:], ALU.is_gt)
                c0 = tpool.tile([128, FP], F32, tag="tmp")
                nc.vector.tensor_tensor(c0[:], y[:], g[:], ALU.subtract)
                m0 = tpool.tile([128, FP], F32, tag="tmp")
                nc.vector.tensor_scalar(m0[:], c0[:], 255.0, None, ALU.mult)
                s1 = tpool.tile([128, FP], F32, tag="tmp")
                nc.vector.tensor_scalar(s1[:], m0[:], 255.0, None, ALU.add)
                aa = tpool.tile([128, FP], F32, tag="tmp")
                nc.vector.tensor_tensor(aa[:], z[:], s1[:], ALU.is_ge)
                b2 = tpool.tile([128, FP], F32, tag="tmp")
                nc.vector.tensor_tensor(b2[:], z[:], m0[:], ALU.is_lt)
                tq = tpool.tile([128, FP], F32, tag="tmp")
                nc.vector.tensor_tensor(tq[:], aa[:], b2[:], ALU.subtract)
                f = tpool.tile([128, FP], F32, tag="tmp")
                nc.vector.tensor_tensor(f[:], c0[:], tq[:], ALU.add)
                bins = tpool.tile([128, FP], F32, tag="tmp")
                nc.vector.tensor_scalar(bins[:], f[:], 255.0, None, ALU.min)
                idx = ipool.tile([128, FP], I16, tag=f"idx{c}")
                # idx = bins + 256*XQ(partition)
                nc.vector.tensor_scalar(idx[:], bins[:], xq[:], None, ALU.add)
                idxt.append(idx)

        tabp = ctx.enter_context(tc.tile_pool(name="tab", bufs=1))
        gpool = ctx.enter_context(tc.tile_pool(name="g", bufs=2))
        g2pool = ctx.enter_context(tc.tile_pool(name="g2", bufs=2))
        apool = ctx.enter_context(tc.tile_pool(name="acc", bufs=2))
        fpool = ctx.enter_context(tc.tile_pool(name="f", bufs=4))

        for hf in range(2):
            tabs = []
            for c in range(C):
                tt = tabp.tile([128, NUM_ELEMS], F32, tag=f"tab{c}")
                nc.sync.dma_start(tt[:], tab.ap()[hf, c])
                tabs.append(tt)
            for c in range(C):
                for jc in range(8):
                    f0 = hf * 2048 + jc * 256
                    gt = gpool.tile([128, 4096], F32, tag="g")
                    nc.gpsimd.ap_gather(gt[:], tabs[c][:], idxt[c][:, f0:f0 + 256],
                                        channels=128, num_elems=NUM_ELEMS, d=1,
                                        num_idxs=4096)
                    # multiply by (1-wy)/wy then (1-wx)/wx  (exact ref order)
                    # gather-out free index j = 16*a + b ; a = 16*row_rel+colin
                    g2 = g2pool.tile([128, 4096], F32, tag="g2")
                    gv = gt[:].rearrange("p (r ci b) -> p r ci b", r=16, ci=16, b=16)
                    g2v = g2[:].rearrange("p (r ci b) -> p r ci b", r=16, ci=16, b=16)
                    wyv = wy[:, hf * 128 + jc * 16: hf * 128 + jc * 16 + 16]
                    wyv = wyv.unsqueeze(2).unsqueeze(3).broadcast_to([128, 16, 16, 16])
                    nc.vector.tensor_tensor(g2v, gv, wyv, ALU.mult)
                    wxv = wx[:].rearrange("p (ci b) -> p ci b", ci=16, b=16)
                    wxv = wxv.unsqueeze(1).broadcast_to([128, 16, 16, 16])
                    nc.vector.tensor_tensor(g2v, g2v, wxv, ALU.mult)
                    # exact 4-slot sum in reference order via 32x32
                    # DVE transpose: slots move to the free dim
                    tr = g2pool.tile([128, 4096], F32, tag="tr")
                    nc.vector.transpose(tr[:], g2[:])
                    trv = tr[:].rearrange("p (J k s) -> p J k s", J=128, k=2, s=16)
                    ac1 = apool.tile([128, 256], F32, tag="ac1")
                    a1v = ac1[:].rearrange("p (k J) -> p J k", k=2)
                    nc.vector.tensor_tensor(a1v, trv[:, :, :, 0], trv[:, :, :, 1],
                                            ALU.add)
                    ac2 = apool.tile([128, 256], F32, tag="ac2")
                    a2v = ac2[:].rearrange("p (k J) -> p J k", k=2)
                    nc.vector.tensor_tensor(a2v, a1v, trv[:, :, :, 2], ALU.add)
                    ac3 = apool.tile([128, 256], F32, tag="ac3")
                    a3v = ac3[:].rearrange("p (k J) -> p J k", k=2)
                    nc.vector.tensor_tensor(a3v, a2v, trv[:, :, :, 3], ALU.add)
                    # scatter to scratch (cp-major inner layout):
                    # scr[c,hf, p_out=32P0+16kp+m, jc*256 + cp*128 + J]
                    for P0 in range(4):
                        for cp in range(2):
                            src = ac3[32 * P0 + 16 * cp:32 * P0 + 16 * cp + 16,
                                      :].rearrange("p (k J) -> p k J", k=2)
                            dst = bass.AP(scr,
                                          (c * 2 + hf) * 262144
                                          + P0 * 65536 + jc * 256 + cp * 128,
                                          [[2048, 16], [32768, 2], [1, 128]])
                            nc.sync.dma_start(dst, src)

        # final: reload scratch in (128, f)-major, exact floor, store
        for c in range(C):
            for hf in range(2):
                r = fpool.tile([128, 2048], F32, tag="f")
                src = bass.AP(scr, (c * 2 + hf) * 262144,
                              [[2048, 128], [1, 2048]])
                nc.sync.dma_start(r[:], src)
                # floor(fl(fl(S/255)*255)) == floor(S) for S in [0,256)
                # (host-verified over the full grid near integers)
                y2 = fpool.tile([128, 2048], F32, tag="f")
                nc.vector.tensor_scalar(y2[:], r[:], MAGIC, MAGIC, ALU.add, ALU.subtract)
                gm = fpool.tile([128, 2048], F32, tag="f")
                nc.vector.tensor_tensor(gm[:], y2[:], r[:], ALU.is_gt)
                res = fpool.tile([128, 2048], F32, tag="f")
                # input layout (jc, cp, J'); write res at jc*256 + 2*J' + cp
                resv = res[:].rearrange("p (j J cp) -> p j cp J",
                                        j=8, J=128, cp=2)
                y2v = y2[:].rearrange("p (j cp J) -> p j cp J", j=8, cp=2, J=128)
                gmv = gm[:].rearrange("p (j cp J) -> p j cp J", j=8, cp=2, J=128)
                nc.vector.tensor_tensor(resv, y2v, gmv, ALU.subtract)
                # res free order: (jc 8, a 256); a = (rowrel 16)*16 + colin
                # pixel: row = hf*128 + jc*16 + rowrel, col = 16*p + colin
                dst = bass.AP(out, c * RH * W + hf * 128 * W,
                              [[16, 128], [16 * W, 8], [W, 16], [1, 16]])
                src = res[:].rearrange("p (j r ci) -> p j r ci", j=8, r=16, ci=16)
                nc.sync.dma_start(dst, src)
    nc.compile()
    return nc


# ---------------- host side ----------------

def _host_bins_check():
    pass


def _exact_luts(cnt_le):
    """cnt_le: (C, GH, GW, 256) float64 exact counts -> luts float32, exact
    replica of the reference clip/cdf math in fp32."""
    f32 = np.float32
    pixels = TH * TW
    hist = np.diff(np.concatenate([np.zeros(cnt_le.shape[:-1] + (1,)), cnt_le],
                                  axis=-1), axis=-1).astype(f32)
    max_val = max(int(40.0 * pixels // NB), 1)
    histos_c = np.minimum(hist, f32(max_val))
    clipped = (f32(pixels) - histos_c.sum(axis=-1, dtype=f32)).astype(f32)
    residual = np.remainder(clipped, f32(NB))
    redist = ((clipped - residual) / f32(NB)).astype(f32)
    histos_c = (histos_c + redist[..., None]).astype(f32)
    histos_c = (histos_c + (np.arange(NB) < residual[..., None]).astype(f32)).astype(f32)
    cum = np.cumsum(histos_c, axis=-1, dtype=f32)
    luts = np.floor(np.clip(cum * f32((NB - 1) / pixels), f32(0.0), f32(NB - 1)))
    return luts.astype(f32)  # (C, GH, GW, 256)


def _weights_host():
    f32 = np.float32
    # exact replica of ys/xs/wy/wx fp32 math
    hs = np.arange(H, dtype=f32)
    ys = (hs + f32(0.5)) / f32(TH) - f32(0.5)
    yf = np.floor(ys)
    wy = (ys - yf).astype(f32)          # (H,)
    ws = np.arange(W, dtype=f32)
    xs = (ws + f32(0.5)) / f32(TW) - f32(0.5)
    xf = np.floor(xs)
    wx = (xs - xf).astype(f32)          # (W,)
    y0 = np.clip(yf.astype(np.int64), 0, GH - 1)
    y1 = np.clip(yf.astype(np.int64) + 1, 0, GH - 1)
    x0 = np.clip(xf.astype(np.int64), 0, GW - 1)
    x1 = np.clip(xf.astype(np.int64) + 1, 0, GW - 1)
    return wy, wx, y0, y1, x0, x1, xf


_CACHE = {}
LAST_EXEC_NS = None
LAST_EXEC_DETAIL = {}
LAST_IN1 = None
LAST_IN2 = None


def _get(name, builder):
    if name not in _CACHE:
        _CACHE[name] = builder()
    return _CACHE[name]


def _run(nc, in_maps, core_ids, tag):
    import os
    global LAST_EXEC_NS
    trace = bool(os.environ.get("BASS_PROFILE"))
    res = run_bass_kernel_spmd(nc, in_maps, core_ids, trace=trace)
    if trace and res.exec_time_ns is not None:
        LAST_EXEC_DETAIL[tag] = res.exec_time_ns
        LAST_EXEC_NS = sum(LAST_EXEC_DETAIL.values())
    return res.results


def kernel(image, label, keypoints, mask, probe):
    f32 = np.float32
    im = np.asarray(image, f32)
    core_ids = list(range(N_CORES))

    # ---- phase 1: histograms ----
    nc1 = _get("p1", build_phase1)
    ab_in = np.tile(-(np.arange(ACT_BIN_START, ACT_BIN_START + 64) + 0.5)
                    .astype(f32), (128, 1))
    in1 = [{"img": np.ascontiguousarray(im[:, RH * r:RH * (r + 1), :]),
            "abias": ab_in}
           for r in range(N_CORES)]
    global LAST_IN1
    LAST_IN1 = in1
    res1 = _run(nc1, in1, core_ids, "p1")

    cnt_le = np.zeros((C, GH, GW, NB), np.float64)
    npart = f32(FP)
    for r in range(N_CORES):
        cd = np.asarray(res1[r]["cntd"], np.float64)  # (C,128,256)
        ca = np.asarray(res1[r]["cnta"], np.float64)
        per_part = np.empty((C, 128, NB), np.float64)
        per_part[:, :, :ACT_BIN_START] = cd[:, :, :ACT_BIN_START]
        per_part[:, :, ACT_BIN_START:NB - 1] = (
            (FP - ca[:, :, ACT_BIN_START:NB - 1]) / 2.0)
        per_part[:, :, NB - 1] = FP
        # partition p -> tile x = p//16
        cnt_le[:, r, :, :] = per_part.reshape(C, GW, 16, NB).sum(axis=2)

    luts = _exact_luts(cnt_le)  # (C, GH, GW, 256) f32

    # ---- host tables for phase 2 ----
    wyv, wxv, y0v, y1v, x0v, x1v, xfv = _weights_host()
    nc2 = _get("p2", build_phase2)

    # xq per partition (shared by all cores): partition p covers cols [16p,16p+16)
    colg = (np.arange(128) * 16)
    xq_part = (np.clip(np.floor((colg + 0.5) / TW - 0.5).astype(np.int64), -1,
                       GW - 1) + 1)          # 0..8, constant within group
    xq_in = (xq_part[:, None] * 256).astype(f32)

    # wx tile (128, 256): value at (p, t): slot = p%4 in {0,2}->1-wx, {1,3}->wx
    # pixel col-in-tile u = 16*(t%16) + (t//16); global col = 256k+u (k-indep)
    t = np.arange(256)
    u = 16 * (t % 16) + (t // 16)
    wx_u = wxv[u]  # frac is k-independent; use k=0 columns
    wx_tile = np.empty((128, 256), f32)
    for p in range(128):
        wx_tile[p] = (f32(1.0) - wx_u) if (p % 4) in (0, 2) else wx_u

    # sm (128, 8): sum slots 0..3 of group k
    sm_in = np.zeros((128, 8), f32)
    for p in range(128):
        if p % 16 < 4:
            sm_in[p, p // 16] = 1.0

    out_full = np.empty((C, H, W), f32)
    in2 = []
    for r in range(N_CORES):
        rows = np.arange(RH * r, RH * (r + 1))
        wy_in = np.empty((2, 128, 128), f32)
        for hf in range(2):
            rr = rows[hf * 128:(hf + 1) * 128]
            for p in range(128):
                wv = wyv[rr]
                wy_in[hf, p] = (f32(1.0) - wv) if (p % 4) in (0, 1) else wv
        tab_in = np.empty((2, C, 128, NUM_ELEMS), f32)
        for hf in range(2):
            row0 = RH * r + hf * 128
            ty0, ty1 = y0v[row0], y1v[row0]
            for cc in range(C):
                for XQ in range(9):
                    xa = min(max(XQ - 1, 0), GW - 1)
                    xb = min(XQ, GW - 1)
                    va0 = luts[cc, ty0, xa]  # A
                    vb0 = luts[cc, ty0, xb]  # B
                    va1 = luts[cc, ty1, xa]  # C
                    vb1 = luts[cc, ty1, xb]  # D
                    for p in range(128):
                        tabv = (va0, vb0, va1, vb1)[p % 4]
                        tab_in[hf, cc, p, XQ * 256:(XQ + 1) * 256] = tabv
        in2.append({"img": in1[r]["img"], "tab": tab_in, "wy": wy_in,
                    "wx": wx_tile, "xq": xq_in, "sm": sm_in})

    global LAST_IN2
    LAST_IN2 = in2
    res2 = _run(nc2, in2, core_ids, "p2")
    for r in range(N_CORES):
        out_full[:, RH * r:RH * (r + 1), :] = res2[r]["out"]

    mk = np.asarray(mask)
    if not (mk == 1.0).all():
        out_full = (out_full * mk.astype(f32)).astype(f32)
    return (out_full, label, keypoints, mask, probe)


# revision 28
# speedup vs baseline: 1.0414x; 1.0414x over previous
"""CLAHE (kornia-style) Trainium2 kernel, 8 NeuronCores, H-sharded.

Phase 1 (device): per-tile cnt_le histograms via fused compare+accumulate
  (DVE tensor_scalar is_le + ACT Sign, split across bins).
Phase 2 (device): bilinear LUT apply via GPSIMD ap_gather with a 4-variant
  table trick (idx = XQ*256 + bin), exact-order bilinear blend (DVE mults,
  PE 4-partition sum), exact floor pipeline.
Host: tiny (192,256) LUT math replicating the reference bit-exactly.
"""
import sys

sys.path.insert(0, "/opt/trn_rl_repo")
from contextlib import ExitStack

import numpy as np

import concourse.bass as bass
import concourse.tile as tile
from concourse import bacc, mybir
from concourse.bass_utils import run_bass_kernel_spmd

F32 = mybir.dt.float32
BF16 = mybir.dt.bfloat16
I16 = mybir.dt.int16
ALU = mybir.AluOpType
ACTF = mybir.ActivationFunctionType

N_CORES = 8
C, H, W = 3, 2048, 2048
GH = GW = 8
TH = TW = 256
NB = 256
RH = H // N_CORES          # rows per core = 256
FP = RH * 16               # free elems per partition per channel = 4096
MAGIC = float(2 ** 23)     # RNE-to-integer trick constant

# bins 0..254 computed on device; split between DVE and ACT
ACT_BIN_START = 194        # bins [194, 255) on ACT, [0, 194) on DVE
NUM_ELEMS = 9 * NB         # gather table entries (XQ in 0..8)


def _img_slice_ap(t, c):
    # dram img (C, RH, W) viewed as (128 part[colgroup], RH rows, 16 colin)
    return bass.AP(t, c * RH * W, [[16, 128], [W, RH], [1, 16]])


def build_phase1():
    nc = bacc.Bacc("TRN2", target_bir_lowering=False, debug=False,
                   num_devices=N_CORES)
    img = nc.dram_tensor("img", [C, RH, W], F32, kind="ExternalInput")
    abias = nc.dram_tensor("abias", [128, 64], F32, kind="ExternalInput")
    cntd = nc.dram_tensor("cntd", [C, 128, NB], F32, kind="ExternalOutput").ap()
    cnta = nc.dram_tensor("cnta", [C, 128, NB], F32, kind="ExternalOutput").ap()

    with tile.TileContext(nc) as tc, ExitStack() as ctx:
        vpool = ctx.enter_context(tc.tile_pool(name="v", bufs=2))
        tpool = ctx.enter_context(tc.tile_pool(name="t", bufs=7))
        bpool = ctx.enter_context(tc.tile_pool(name="b", bufs=2))
        jpool = ctx.enter_context(tc.tile_pool(name="j", bufs=2))
        cpool = ctx.enter_context(tc.tile_pool(name="c", bufs=2))
        kpool = ctx.enter_context(tc.tile_pool(name="k", bufs=1))
        ab = kpool.tile([128, 64], F32)
        nc.sync.dma_start(ab[:], abias.ap())
        for c in range(C):
            v = vpool.tile([128, FP], F32)
            nc.sync.dma_start(v[:], _img_slice_ap(img, c))
            # exact bins without divide: bins = #{b>=1: 256*v >= 255*b}
            # (min{v: fl(v/255) >= b/256} == 255b/256 exactly, host-verified)
            z = tpool.tile([128, FP], F32, tag="tmp")
            nc.vector.tensor_scalar(z[:], v[:], 256.0, None, ALU.mult)
            w = tpool.tile([128, FP], F32, tag="tmp")
            nc.vector.tensor_scalar(w[:], z[:], float(np.float32(1.0) / np.float32(255.0)), None, ALU.mult)
            y = tpool.tile([128, FP], F32, tag="tmp")
            nc.vector.tensor_scalar(y[:], w[:], MAGIC, MAGIC, ALU.add, ALU.subtract)
            g = tpool.tile([128, FP], F32, tag="tmp")
            nc.vector.tensor_tensor(g[:], y[:], w[:], ALU.is_gt)
            c0 = tpool.tile([128, FP], F32, tag="tmp")
            nc.vector.tensor_tensor(c0[:], y[:], g[:], ALU.subtract)
            m0 = tpool.tile([128, FP], F32, tag="tmp")
            nc.vector.tensor_scalar(m0[:], c0[:], 255.0, None, ALU.mult)
            s1 = tpool.tile([128, FP], F32, tag="tmp")
            nc.vector.tensor_scalar(s1[:], m0[:], 255.0, None, ALU.add)
            aa = tpool.tile([128, FP], F32, tag="tmp")
            nc.vector.tensor_tensor(aa[:], z[:], s1[:], ALU.is_ge)
            b2 = tpool.tile([128, FP], F32, tag="tmp")
            nc.vector.tensor_tensor(b2[:], z[:], m0[:], ALU.is_lt)
            tq = tpool.tile([128, FP], F32, tag="tmp")
            nc.vector.tensor_tensor(tq[:], aa[:], b2[:], ALU.subtract)
            f = tpool.tile([128, FP], F32, tag="tmp")
            nc.vector.tensor_tensor(f[:], c0[:], tq[:], ALU.add)
            bins = bpool.tile([128, FP], BF16)
            nc.vector.tensor_scalar(bins[:], f[:], 255.0, None, ALU.min)

            cd = cpool.tile([128, NB], F32, tag="cd")
            ca = cpool.tile([128, NB], F32, tag="ca")
            junkd = jpool.tile([128, FP], BF16, tag="jd")
            junka = jpool.tile([128, FP], BF16, tag="ja")
            for b in range(ACT_BIN_START):
                nc.vector.tensor_scalar(junkd[:], bins[:], b + 0.5, None,
                                        ALU.is_le, ALU.add,
                                        accum_out=cd[:, b:b + 1])
            for b in range(ACT_BIN_START, NB - 1):
                i = b - ACT_BIN_START
                nc.scalar.activation(junka[:], bins[:], ACTF.Sign,
                                     bias=ab[:, i:i + 1], scale=1.0,
                                     accum_out=ca[:, b:b + 1])
            nc.sync.dma_start(cntd[c][:, :ACT_BIN_START],
                              cd[:, :ACT_BIN_START])
            nc.sync.dma_start(cnta[c][:, ACT_BIN_START:NB - 1],
                              ca[:, ACT_BIN_START:NB - 1])
    nc.compile()
    return nc


def build_phase2():
    nc = bacc.Bacc("TRN2", target_bir_lowering=False, debug=False,
                   num_devices=N_CORES)
    img = nc.dram_tensor("img", [C, RH, W], F32, kind="ExternalInput")
    tab = nc.dram_tensor("tab", [2, C, 128, NUM_ELEMS], F32, kind="ExternalInput")
    wyt = nc.dram_tensor("wy", [2, 128, 128], F32, kind="ExternalInput")
    wxt = nc.dram_tensor("wx", [128, 256], F32, kind="ExternalInput")
    xqt = nc.dram_tensor("xq", [128, 1], F32, kind="ExternalInput")
    smt = nc.dram_tensor("sm", [128, 8], F32, kind="ExternalInput")
    out = nc.dram_tensor("out", [C, RH, W], F32, kind="ExternalOutput")
    scr = nc.dram_tensor("scr", [C, 2, 128, 2048], F32)  # output layout

    with tile.TileContext(nc) as tc, ExitStack() as ctx:
        ipool = ctx.enter_context(tc.tile_pool(name="i", bufs=1))
        kpool = ctx.enter_context(tc.tile_pool(name="k", bufs=1))

        # resident constants
        wy = kpool.tile([128, 2 * 128], F32)
        nc.sync.dma_start(wy[:], bass.AP(wyt, 0, [[128, 128], [16384, 2], [1, 128]]))
        wx = kpool.tile([128, 256], F32)
        nc.sync.dma_start(wx[:], wxt.ap())
        xq = kpool.tile([128, 1], F32)
        nc.sync.dma_start(xq[:], xqt.ap())
        # per-channel bins -> idx (int16), resident
        idxt = []
        with ExitStack() as prep:
            vpool = prep.enter_context(tc.tile_pool(name="v", bufs=2))
            tpool = prep.enter_context(tc.tile_pool(name="t", bufs=7))
            for c in range(C):
                v = vpool.tile([128, FP], F32)
                nc.sync.dma_start(v[:], _img_slice_ap(img, c))
                z = tpool.tile([128, FP], F32, tag="tmp")
                nc.vector.tensor_scalar(z[:], v[:], 256.0, None, ALU.mult)
                w = tpool.tile([128, FP], F32, tag="tmp")
                nc.vector.tensor_scalar(w[:], z[:], float(np.float32(1.0) / np.float32(255.0)), None, ALU.mult)
                y = tpool.tile([128, FP], F32, tag="tmp")
                nc.vector.tensor_scalar(y[:], w[:], MAGIC, MAGIC, ALU.add, ALU.subtract)
                g = tpool.tile([128, FP], F32, tag="tmp")
                nc.vector.tensor_tensor(g[:], y[:], w[:], ALU.is_gt)
                c0 = tpool.tile([128, FP], F32, tag="tmp")
                nc.vector.tensor_tensor(c0[:], y[:], g[:], ALU.subtract)
                m0 = tpool.tile([128, FP], F32, tag="tmp")
                nc.vector.tensor_scalar(m0[:], c0[:], 255.0, None, ALU.mult)
                s1 = tpool.tile([128, FP], F32, tag="tmp")
                nc.vector.tensor_scalar(s1[:], m0[:], 255.0, None, ALU.add)
                aa = tpool.tile([128, FP], F32, tag="tmp")
                nc.vector.tensor_tensor(aa[:], z[:], s1[:], ALU.is_ge)
                b2 = tpool.tile([128, FP], F32, tag="tmp")
                nc.vector.tensor_tensor(b2[:], z[:], m0[:], ALU.is_lt)
                tq = tpool.tile([128, FP], F32, tag="tmp")
                nc.vector.tensor_tensor(tq[:], aa[:], b2[:], ALU.subtract)
                f = tpool.tile([128, FP], F32, tag="tmp")
                nc.vector.tensor_tensor(f[:], c0[:], tq[:], ALU.add)
                bins = tpool.tile([128, FP], F32, tag="tmp")
                nc.vector.tensor_scalar(bins[:], f[:], 255.0, None, ALU.min)
                idx = ipool.tile([128, FP], I16, tag=f"idx{c}")
                # idx = bins + 256*XQ(partition)
                nc.vector.tensor_scalar(idx[:], bins[:], xq[:], None, ALU.add)
                idxt.append(idx)

        tabp = ctx.enter_context(tc.tile_pool(name="tab", bufs=1))
        gpool = ctx.enter_context(tc.tile_pool(name="g", bufs=3))
        g2pool = ctx.enter_context(tc.tile_pool(name="g2", bufs=2))
        apool = ctx.enter_context(tc.tile_pool(name="acc", bufs=2))
        fpool = ctx.enter_context(tc.tile_pool(name="f", bufs=4))

        for hf in range(2):
            tabs = []
            for c in range(C):
                tt = tabp.tile([128, NUM_ELEMS], F32, tag=f"tab{c}")
                nc.sync.dma_start(tt[:], tab.ap()[hf, c])
                tabs.append(tt)
            for c in range(C):
                for jc in range(8):
                    f0 = hf * 2048 + jc * 256
                    gt = gpool.tile([128, 4096], F32, tag="g")
                    nc.gpsimd.ap_gather(gt[:], tabs[c][:], idxt[c][:, f0:f0 + 256],
                                        channels=128, num_elems=NUM_ELEMS, d=1,
                                        num_idxs=4096)
                    # multiply by (1-wy)/wy then (1-wx)/wx  (exact ref order)
                    # gather-out free index j = 16*a + b ; a = 16*row_rel+colin
                    g2 = g2pool.tile([128, 4096], F32, tag="g2")
                    gv = gt[:].rearrange("p (r ci b) -> p r ci b", r=16, ci=16, b=16)
                    g2v = g2[:].rearrange("p (r ci b) -> p r ci b", r=16, ci=16, b=16)
                    wyv = wy[:, hf * 128 + jc * 16: hf * 128 + jc * 16 + 16]
                    wyv = wyv.unsqueeze(2).unsqueeze(3).broadcast_to([128, 16, 16, 16])
                    nc.vector.tensor_tensor(g2v, gv, wyv, ALU.mult)
                    wxv = wx[:].rearrange("p (ci b) -> p ci b", ci=16, b=16)
                    wxv = wxv.unsqueeze(1).broadcast_to([128, 16, 16, 16])
                    nc.vector.tensor_tensor(g2v, g2v, wxv, ALU.mult)
                    # exact 4-slot sum in reference order via 32x32
                    # DVE transpose: slots move to the free dim
                    tr = g2pool.tile([128, 4096], F32, tag="tr")
                    nc.vector.transpose(tr[:], g2[:])
                    trv = tr[:].rearrange("p (J k s) -> p J k s", J=128, k=2, s=16)
                    ac1 = apool.tile([128, 256], F32, tag="ac1")
                    a1v = ac1[:].rearrange("p (k J) -> p J k", k=2)
                    nc.vector.tensor_tensor(a1v, trv[:, :, :, 0], trv[:, :, :, 1],
                                            ALU.add)
                    ac2 = apool.tile([128, 256], F32, tag="ac2")
                    a2v = ac2[:].rearrange("p (k J) -> p J k", k=2)
                    nc.vector.tensor_tensor(a2v, a1v, trv[:, :, :, 2], ALU.add)
                    ac3 = apool.tile([128, 256], F32, tag="ac3")
                    a3v = ac3[:].rearrange("p (k J) -> p J k", k=2)
                    nc.vector.tensor_tensor(a3v, a2v, trv[:, :, :, 3], ALU.add)
                    # scatter to scratch (cp-major inner layout):
                    # scr[c,hf, p_out=32P0+16kp+m, jc*256 + cp*128 + J]
                    for P0 in range(4):
                        for cp in range(2):
                            src = ac3[32 * P0 + 16 * cp:32 * P0 + 16 * cp + 16,
                                      :].rearrange("p (k J) -> p k J", k=2)
                            dst = bass.AP(scr,
                                          (c * 2 + hf) * 262144
                                          + P0 * 65536 + jc * 256 + cp * 128,
                                          [[2048, 16], [32768, 2], [1, 128]])
                            nc.sync.dma_start(dst, src)

        # final: reload scratch in (128, f)-major, exact floor, store
        for c in range(C):
            for hf in range(2):
                r = fpool.tile([128, 2048], F32, tag="f")
                src = bass.AP(scr, (c * 2 + hf) * 262144,
                              [[2048, 128], [1, 2048]])
                nc.sync.dma_start(r[:], src)
                # floor(fl(fl(S/255)*255)) == floor(S) for S in [0,256)
                # (host-verified over the full grid near integers)
                y2 = fpool.tile([128, 2048], F32, tag="f")
                nc.vector.tensor_scalar(y2[:], r[:], MAGIC, MAGIC, ALU.add, ALU.subtract)
                gm = fpool.tile([128, 2048], F32, tag="f")
                nc.vector.tensor_tensor(gm[:], y2[:], r[:], ALU.is_gt)
                res = fpool.tile([128, 2048], F32, tag="f")
                # input layout (jc, cp, J'); write res at jc*256 + 2*J' + cp
                resv = res[:].rearrange("p (j J cp) -> p j cp J",
                                        j=8, J=128, cp=2)
                y2v = y2[:].rearrange("p (j cp J) -> p j cp J", j=8, cp=2, J=128)
                gmv = gm[:].rearrange("p (j cp J) -> p j cp J", j=8, cp=2, J=128)
                nc.vector.tensor_tensor(resv, y2v, gmv, ALU.subtract)
                # res free order: (jc 8, a 256); a = (rowrel 16)*16 + colin
                # pixel: row = hf*128 + jc*16 + rowrel, col = 16*p + colin
                dst = bass.AP(out, c * RH * W + hf * 128 * W,
                              [[16, 128], [16 * W, 8], [W, 16], [1, 16]])
                src = res[:].rearrange("p (j r ci) -> p j r ci", j=8, r=16, ci=16)
                nc.sync.dma_start(dst, src)
    nc.compile()
    return nc


# ---------------- host side ----------------

def _host_bins_check():
    pass


def _exact_luts(cnt_le):
    """cnt_le: (C, GH, GW, 256) float64 exact counts -> luts float32, exact
    replica of the reference clip/cdf math in fp32."""
    f32 = np.float32
    pixels = TH * TW
    hist = np.diff(np.concatenate([np.zeros(cnt_le.shape[:-1] + (1,)), cnt_le],
                                  axis=-1), axis=-1).astype(f32)
    max_val = max(int(40.0 * pixels // NB), 1)
    histos_c = np.minimum(hist, f32(max_val))
    clipped = (f32(pixels) - histos_c.sum(axis=-1, dtype=f32)).astype(f32)
    residual = np.remainder(clipped, f32(NB))
    redist = ((clipped - residual) / f32(NB)).astype(f32)
    histos_c = (histos_c + redist[..., None]).astype(f32)
    histos_c = (histos_c + (np.arange(NB) < residual[..., None]).astype(f32)).astype(f32)
    cum = np.cumsum(histos_c, axis=-1, dtype=f32)
    luts = np.floor(np.clip(cum * f32((NB - 1) / pixels), f32(0.0), f32(NB - 1)))
    return luts.astype(f32)  # (C, GH, GW, 256)


def _weights_host():
    f32 = np.float32
    # exact replica of ys/xs/wy/wx fp32 math
    hs = np.arange(H, dtype=f32)
    ys = (hs + f32(0.5)) / f32(TH) - f32(0.5)
    yf = np.floor(ys)
    wy = (ys - yf).astype(f32)          # (H,)
    ws = np.arange(W, dtype=f32)
    xs = (ws + f32(0.5)) / f32(TW) - f32(0.5)
    xf = np.floor(xs)
    wx = (xs - xf).astype(f32)          # (W,)
    y0 = np.clip(yf.astype(np.int64), 0, GH - 1)
    y1 = np.clip(yf.astype(np.int64) + 1, 0, GH - 1)
    x0 = np.clip(xf.astype(np.int64), 0, GW - 1)
    x1 = np.clip(xf.astype(np.int64) + 1, 0, GW - 1)
    return wy, wx, y0, y1, x0, x1, xf


_CACHE = {}
LAST_EXEC_NS = None
LAST_EXEC_DETAIL = {}
LAST_IN1 = None
LAST_IN2 = None


def _get(name, builder):
    if name not in _CACHE:
        _CACHE[name] = builder()
    return _CACHE[name]


def _run(nc, in_maps, core_ids, tag):
    import os
    global LAST_EXEC_NS
    trace = bool(os.environ.get("BASS_PROFILE"))
    res = run_bass_kernel_spmd(nc, in_maps, core_ids, trace=trace)
    if trace and res.exec_time_ns is not None:
        LAST_EXEC_DETAIL[tag] = res.exec_time_ns
        LAST_EXEC_NS = sum(LAST_EXEC_DETAIL.values())
    return res.results


def kernel(image, label, keypoints, mask, probe):
    f32 = np.float32
    im = np.asarray(image, f32)
    core_ids = list(range(N_CORES))

    # ---- phase 1: histograms ----
    nc1 = _get("p1", build_phase1)
    ab_in = np.tile(-(np.arange(ACT_BIN_START, ACT_BIN_START + 64) + 0.5)
                    .astype(f32), (128, 1))
    in1 = [{"img": np.ascontiguousarray(im[:, RH * r:RH * (r + 1), :]),
            "abias": ab_in}
           for r in range(N_CORES)]
    global LAST_IN1
    LAST_IN1 = in1
    res1 = _run(nc1, in1, core_ids, "p1")

    cnt_le = np.zeros((C, GH, GW, NB), np.float64)
    npart = f32(FP)
    for r in range(N_CORES):
        cd = np.asarray(res1[r]["cntd"], np.float64)  # (C,128,256)
        ca = np.asarray(res1[r]["cnta"], np.float64)
        per_part = np.empty((C, 128, NB), np.float64)
        per_part[:, :, :ACT_BIN_START] = cd[:, :, :ACT_BIN_START]
        per_part[:, :, ACT_BIN_START:NB - 1] = (
            (FP - ca[:, :, ACT_BIN_START:NB - 1]) / 2.0)
        per_part[:, :, NB - 1] = FP
        # partition p -> tile x = p//16
        cnt_le[:, r, :, :] = per_part.reshape(C, GW, 16, NB).sum(axis=2)

    luts = _exact_luts(cnt_le)  # (C, GH, GW, 256) f32

    # ---- host tables for phase 2 ----
    wyv, wxv, y0v, y1v, x0v, x1v, xfv = _weights_host()
    nc2 = _get("p2", build_phase2)

    # xq per partition (shared by all cores): partition p covers cols [16p,16p+16)
    colg = (np.arange(128) * 16)
    xq_part = (np.clip(np.floor((colg + 0.5) / TW - 0.5).astype(np.int64), -1,
                       GW - 1) + 1)          # 0..8, constant within group
    xq_in = (xq_part[:, None] * 256).astype(f32)

    # wx tile (128, 256): value at (p, t): slot = p%4 in {0,2}->1-wx, {1,3}->wx
    # pixel col-in-tile u = 16*(t%16) + (t//16); global col = 256k+u (k-indep)
    t = np.arange(256)
    u = 16 * (t % 16) + (t // 16)
    wx_u = wxv[u]  # frac is k-independent; use k=0 columns
    wx_tile = np.empty((128, 256), f32)
    for p in range(128):
        wx_tile[p] = (f32(1.0) - wx_u) if (p % 4) in (0, 2) else wx_u

    # sm (128, 8): sum slots 0..3 of group k
    sm_in = np.zeros((128, 8), f32)
    for p in range(128):
        if p % 16 < 4:
            sm_in[p, p // 16] = 1.0

    out_full = np.empty((C, H, W), f32)
    pidx = np.arange(128)
    xa_t = np.minimum(np.maximum(np.arange(9) - 1, 0), GW - 1)   # A-variant x
    xb_t = np.minimum(np.arange(9), GW - 1)                      # B-variant x
    in2 = []
    for r in range(N_CORES):
        rows = np.arange(RH * r, RH * (r + 1))
        wy_in = np.empty((2, 128, 128), f32)
        for hf in range(2):
            wv = wyv[rows[hf * 128:(hf + 1) * 128]]
            wy_in[hf] = np.where((pidx % 4 < 2)[:, None], f32(1.0) - wv, wv)
        tab_in = np.empty((2, C, 128, NUM_ELEMS), f32)
        for hf in range(2):
            row0 = RH * r + hf * 128
            ty0, ty1 = y0v[row0], y1v[row0]
            # variants per slot: (y, xsel): 0:(y0,A) 1:(y0,B) 2:(y1,A) 3:(y1,B)
            vt = np.stack([
                luts[:, ty0][:, xa_t], luts[:, ty0][:, xb_t],
                luts[:, ty1][:, xa_t], luts[:, ty1][:, xb_t],
            ])  # (4, C, 9, 256)
            tab_in[hf] = vt[pidx % 4].reshape(128, C, 9 * NB).transpose(1, 0, 2)
        in2.append({"img": in1[r]["img"], "tab": tab_in, "wy": wy_in,
                    "wx": wx_tile, "xq": xq_in, "sm": sm_in})

    global LAST_IN2
    LAST_IN2 = in2
    res2 = _run(nc2, in2, core_ids, "p2")
    for r in range(N_CORES):
        out_full[:, RH * r:RH * (r + 1), :] = res2[r]["out"]

    mk = np.asarray(mask)
    if not (mk == 1.0).all():
        out_full = (out_full * mk.astype(f32)).astype(f32)
    return (out_full, label, keypoints, mask, probe)
